# revision 1
# baseline (speedup 1.0000x reference)
"""BitLinear-1.58 inference kernel for Trainium2 (8 NeuronCores, token-parallel).

out = (clip(round(x * 128/gamma), -128, 127) @ W^T) * (scale*gamma/128) + bias
with gamma = max(|x|, axis=-1), W ternary {-1,0,1}.

Matmul runs in bf16 which is bit-exact here: xq in [-128,127] and W in
{-1,0,1} are exactly representable in bf16, accumulation is fp32 in PSUM.
"""

import os
import numpy as np
import ml_dtypes
from contextlib import ExitStack


def _env(k, d):
    return int(os.environ.get(k, d))

import concourse.bass as bass
import concourse.mybir as mybir
import concourse.tile as tile
from concourse import bacc
from concourse.bass_utils import run_bass_kernel_spmd

N_CORES = 8
B, S, D_IN, D_OUT = 4, 4096, 2048, 2048
TOKENS = B * S                 # 16384
TPC = TOKENS // N_CORES        # 2048 tokens per core
P = 128
N_TILES = TPC // P             # 16 token tiles per core
KC = D_IN // P                 # 16 contraction chunks
NF = 512                       # matmul free dim (one PSUM bank of fp32)
OC = D_OUT // NF               # 4 output chunks
MAGIC = 12582912.0             # 1.5 * 2**23  (round-half-even trick)
EPS = 1e-5
Q = 128.0

F32 = mybir.dt.float32
BF16 = mybir.dt.bfloat16
F8 = mybir.dt.float8e4
AX = mybir.AxisListType
OP = mybir.AluOpType
AF = mybir.ActivationFunctionType


def build_kernel(n_tiles=N_TILES):
    nc = bacc.Bacc(
        "TRN2", target_bir_lowering=False, debug=False, num_devices=N_CORES
    )
    tpc = n_tiles * P
    x_d = nc.dram_tensor("x", [tpc, D_IN], F32, kind="ExternalInput").ap()
    w_d = nc.dram_tensor("w", [P, KC * D_OUT], F8, kind="ExternalInput").ap()
    b_d = nc.dram_tensor("bias", [P, D_OUT], F32, kind="ExternalInput").ap()
    s_d = nc.dram_tensor("scale", [P, 1], F32, kind="ExternalInput").ap()
    o_d = nc.dram_tensor("out", [tpc, D_OUT], F32, kind="ExternalOutput").ap()

    with tile.TileContext(nc) as tc:
        with ExitStack() as ctx:
            _emit(ctx, tc, o_d, x_d, w_d, b_d, s_d, n_tiles)
    _dedup_ldweights(nc)
    nc.compile()
    return nc


def _dedup_ldweights(nc):
    """Drop InstLdweights whose weights AP matches the previous LDW in the
    same block (PE stationary registers still hold those weights). Waits on a
    dropped LDW are merged into the next kept PE instruction."""
    n_removed = 0
    for bb in nc.main_func.blocks:
        kept = []
        last_key = None
        pending_waits = []
        for inst in bb.instructions:
            if isinstance(inst, mybir.InstLdweights):
                key = repr(inst.ins)
                if key == last_key:
                    si = inst.sync_info
                    if si is not None and si.on_wait:
                        pending_waits.extend(si.on_wait)
                    n_removed += 1
                    continue
                last_key = key
            elif isinstance(inst, (mybir.InstMatmult, mybir.InstEventSemaphore)):
                pass  # does not clobber PE stationary weights
            elif getattr(inst, "engine", None) == mybir.EngineType.PE:
                last_key = None  # conservative: other PE instruction
            if pending_waits and getattr(inst, "engine", None) == mybir.EngineType.PE:
                si = inst.sync_info
                if si is None:
                    inst.sync_info = mybir.SyncInfo(
                        on_wait=list(pending_waits), on_update=[]
                    )
                else:
                    si.on_wait = list(si.on_wait) + pending_waits
                pending_waits = []
            kept.append(inst)
        assert not pending_waits, "dangling waits from dropped trailing LDW"
        bb.instructions[:] = kept
    return n_removed


def _emit(ctx, tc, o_d, x_d, w_d, b_d, s_d, n_tiles):
    nc = tc.nc

    const = ctx.enter_context(tc.tile_pool(name="const", bufs=1))
    xp = ctx.enter_context(tc.tile_pool(name="xp", bufs=_env("K_XP", 3)))
    tp = ctx.enter_context(tc.tile_pool(name="tp", bufs=_env("K_TP", 2)))
    qp = ctx.enter_context(tc.tile_pool(name="qp", bufs=_env("K_QP", 2)))
    qtp = ctx.enter_context(tc.tile_pool(name="qtp", bufs=_env("K_QTP", 2)))
    outp = ctx.enter_context(tc.tile_pool(name="outp", bufs=_env("K_OUTP", 2)))
    smp = ctx.enter_context(tc.tile_pool(name="smp", bufs=_env("K_SMP", 2)))
    psp = ctx.enter_context(tc.tile_pool(name="psp", bufs=2, space="PSUM"))

    # prefetch the first two x tiles; tile 0 arrives as halves
    HD = D_IN // 2
    x_pre = []
    for i in range(min(_env("K_XPRE", 2), n_tiles)):
        x_t = xp.tile([P, D_IN], F32, tag="x", name=f"x_pre{i}")
        if i < _env("K_SPLITN", 8) and _env("K_SPLIT0", 1):
            r = slice(i * P, (i + 1) * P)
            nc.sync.dma_start(x_t[:, :HD], x_d[r, :HD])
            nc.sync.dma_start(x_t[:, HD:], x_d[r, HD:])
        else:
            nc.sync.dma_start(x_t[:], x_d[i * P : (i + 1) * P, :])
        x_pre.append(x_t)

    magic_sb = const.tile([P, 1], F32)
    nc.any.memset(magic_sb[:], MAGIC)
    # touch ScalarE once so its activation table load runs during startup fill
    warm_act = const.tile([P, 1], F32)
    nc.scalar.activation(warm_act[:], magic_sb[:], AF.Identity, bias=magic_sb[:, 0:1])
    scale_sb = const.tile([P, 1], F32)
    nc.sync.dma_start(scale_sb[:], s_d[:])

    # chunked fp8 weight load (ternary weights are exact in fp8) + upcast to
    # bf16 split across ScalarE and GpSimd: HBM ships 4MB instead of 8MB
    w_sb = const.tile([P, KC * D_OUT], BF16)
    w8p = ctx.enter_context(tc.tile_pool(name="w8p", bufs=_env("K_W8B", 4)))

    def load_w_chunk(c):
        w8 = w8p.tile([P, D_OUT], F8, tag="w8", name=f"w8_{c}")
        nc.sync.dma_start(w8[:], w_d[:, c * D_OUT : (c + 1) * D_OUT])
        if c % 2 == _env("K_UPAR", 0):
            nc.gpsimd.tensor_copy(w_sb[:, c * D_OUT : (c + 1) * D_OUT], w8[:])
        else:
            nc.scalar.copy(w_sb[:, c * D_OUT : (c + 1) * D_OUT], w8[:])

    for c in range(_env("K_PRE", 4)):
        load_w_chunk(c)
    bias_sb = const.tile([P, D_OUT], F32)

    for i in range(n_tiles):
        if i < len(x_pre):
            x_t = x_pre[i]
        else:
            x_t = xp.tile([P, D_IN], F32, tag="x")
            nc.sync.dma_start(x_t[:], x_d[i * P : (i + 1) * P, :])

        # gamma = max(|x|) per token; g2 = max(gamma, eps)/128
        gamma = smp.tile([P, 1], F32, tag="gamma")
        if i < _env("K_SPLITN", 8) and _env("K_SPLIT0", 1):
            ga = smp.tile([P, 1], F32, tag="ga")
            nc.vector.tensor_reduce(
                ga[:], x_t[:, :HD], axis=AX.X, op=OP.max, apply_absolute_value=True
            )
            gb = smp.tile([P, 1], F32, tag="gb")
            nc.vector.tensor_reduce(
                gb[:], x_t[:, HD:], axis=AX.X, op=OP.max, apply_absolute_value=True
            )
            nc.vector.tensor_max(gamma[:], ga[:], gb[:])
        else:
            nc.vector.tensor_reduce(
                gamma[:], x_t[:], axis=AX.X, op=OP.max, apply_absolute_value=True
            )
        g2 = smp.tile([P, 1], F32, tag="g2")
        nc.vector.tensor_scalar(g2[:], gamma[:], EPS, 1.0 / Q, OP.max, OP.mult)
        # inv = 1/g2 = 128/gamma ; deq = g2*scale = gamma*scale/128
        inv = smp.tile([P, 1], F32, tag="inv")
        nc.vector.reciprocal(inv[:], g2[:])
        deq = smp.tile([P, 1], F32, tag="deq")
        nc.vector.tensor_scalar(deq[:], g2[:], scale_sb[:, 0:1], None, OP.mult)

        # t1 = x*inv + MAGIC  (ScalarE; per-partition scale).  t1 - MAGIC is
        # round-half-even(x*128/gamma), always >= -128, so only the high clip
        # is needed; integers <= 128 are exact in bf16.
        t1 = tp.tile([P, D_IN], F32, tag="t1")
        xq = qp.tile([P, D_IN], BF16, tag="xq")
        xqT = qtp.tile([P, D_IN], BF16, tag="xqT")
        xqT3 = xqT.rearrange("p (c t) -> p c t", c=KC)
        if i == 0 and _env("K_SPLIT0", 1) >= 2:
            for h in range(2):
                cs = slice(h * HD, (h + 1) * HD)
                nc.scalar.activation(
                    t1[:, cs], x_t[:, cs], AF.Identity,
                    bias=magic_sb[:, 0:1], scale=inv[:, 0:1],
                )
                nc.vector.tensor_scalar(
                    xq[:, cs], t1[:, cs], MAGIC, Q - 1.0, OP.subtract, OP.min
                )
                hk2 = KC // 2
                nc.sync.dma_start_transpose(
                    xqT3[:, h * hk2 : (h + 1) * hk2, :], xq[:, cs]
                )
        else:
            nc.scalar.activation(
                t1[:], x_t[:], AF.Identity, bias=magic_sb[:, 0:1], scale=inv[:, 0:1]
            )
            nc.vector.tensor_scalar(xq[:], t1[:], MAGIC, Q - 1.0, OP.subtract, OP.min)
            nsp = _env("K_TSPLIT", 2)
            hk = KC // nsp
            for sp in range(nsp):
                nc.sync.dma_start_transpose(
                    xqT3[:, sp * hk : (sp + 1) * hk, :],
                    xq[:, sp * hk * P : (sp + 1) * hk * P],
                )

        if i == 0:
            # rest of the weight load, after tile 0's transpose is queued
            for c in range(_env("K_PRE", 4), KC):
                load_w_chunk(c)
            nc.sync.dma_start(bias_sb[:], b_d[:])

        # out[t, o] = sum_d xqT[d, t] * wT[d, o]
        ps = psp.tile([P, D_OUT], F32, tag="ps")
        last_ocouter = i == n_tiles - 1 and _env("K_LASTOC", 0)
        if last_ocouter:
            # drain pipeline: finish each PSUM bank early so its dequant and
            # store overlap the remaining matmuls
            o_t = outp.tile([P, D_OUT], F32, tag="o", name="o_last")
            for oc in range(OC):
                for c in range(KC):
                    nc.tensor.matmul(
                        ps[:, oc * NF : (oc + 1) * NF],
                        xqT[:, c * P : (c + 1) * P],
                        w_sb[:, c * D_OUT + oc * NF : c * D_OUT + (oc + 1) * NF],
                        start=(c == 0),
                        stop=(c == KC - 1),
                    )
                sl = slice(oc * NF, (oc + 1) * NF)
                nc.vector.scalar_tensor_tensor(
                    o_t[:, sl], ps[:, sl], deq[:, 0:1], bias_sb[:, sl],
                    OP.mult, OP.add,
                )
                nc.sync.dma_start(o_d[i * P : (i + 1) * P, sl], o_t[:, sl])
            continue
        for c in range(KC):
            lhsT = xqT[:, c * P : (c + 1) * P]
            for oc in range(OC):
                nc.tensor.matmul(
                    ps[:, oc * NF : (oc + 1) * NF],
                    lhsT,
                    w_sb[:, c * D_OUT + oc * NF : c * D_OUT + (oc + 1) * NF],
                    start=(c == 0),
                    stop=(c == KC - 1),
                )

        # dequant + bias in 2 chunks, each followed by its store
        o_t = outp.tile([P, D_OUT], F32, tag="o")
        n_chunks = _env("K_LASTC", 4) if i == n_tiles - 1 else _env("K_OC", 2)
        half = D_OUT // n_chunks
        for hc in range(n_chunks):
            sl = slice(hc * half, (hc + 1) * half)
            nc.vector.scalar_tensor_tensor(
                o_t[:, sl], ps[:, sl], deq[:, 0:1], bias_sb[:, sl], OP.mult, OP.add
            )
            nc.sync.dma_start(o_d[i * P : (i + 1) * P, sl], o_t[:, sl])


def prep_inputs(x, quantized_weight, scale, bias):
    x = np.asarray(x, dtype=np.float32)
    quantized_weight = np.asarray(quantized_weight, dtype=np.float32)
    scale = np.asarray(scale, dtype=np.float32)
    bias = np.asarray(bias, dtype=np.float32)
    xf = np.ascontiguousarray(x.reshape(-1, D_IN))
    wT = quantized_weight.T.astype(ml_dtypes.float8_e4m3fn)  # [d, o], exact ternary
    w_prep = np.ascontiguousarray(
        wT.reshape(KC, P, D_OUT).transpose(1, 0, 2).reshape(P, KC * D_OUT)
    )
    bias_bc = np.ascontiguousarray(
        np.broadcast_to(bias.astype(np.float32), (P, D_OUT))
    )
    scale_bc = np.full((P, 1), np.float32(scale), dtype=np.float32)
    return xf, w_prep, bias_bc, scale_bc


_NC_CACHE = {}


def get_nc(n_tiles=N_TILES):
    if n_tiles not in _NC_CACHE:
        _NC_CACHE[n_tiles] = build_kernel(n_tiles)
    return _NC_CACHE[n_tiles]


def kernel(x, quantized_weight, scale, bias, _trace=False):
    xf, w_prep, bias_bc, scale_bc = prep_inputs(x, quantized_weight, scale, bias)
    in_maps = [
        {
            "x": xf[i * TPC : (i + 1) * TPC],
            "w": w_prep,
            "bias": bias_bc,
            "scale": scale_bc,
        }
        for i in range(N_CORES)
    ]
    nc = get_nc()
    res = run_bass_kernel_spmd(nc, in_maps, list(range(N_CORES)), trace=_trace)
    out = np.concatenate([res.results[i]["out"] for i in range(N_CORES)], axis=0)
    out = out.reshape(B, S, D_OUT).astype(np.float32)
    if _trace:
        return out, res
    return out

